# revision 34
# baseline (speedup 1.0000x reference)
"""BehaviorAwareGCNLayer on 8 Trainium2 NeuronCores.

Math (reference):
    hx  = x @ W
    out[r] = (1/deg[r]) * sum_{e: row[e]=r} sim_w[e]*sigmoid(rep[row]+rep[col])*ns[col] * hx[col]
    out += sigmoid(rep) * (x @ W_self);  leaky_relu(out, 0.01)

Device strategy (destination sharding, no collectives):
  - By linearity, W is applied AFTER aggregation: agg[r] = sum coef_e * x[col_e],
    out[r] = (agg[r]/deg[r]) @ W + sigmoid(rep_r)*(x_r @ W_self).
  - Host does LAYOUT only (grouping/padding/fancy-index staging): it stages
    the per-edge x[col] rows and per-edge scalars (rep[row], rep[col], sw,
    ns[col]) into slot order so the device reads fully sequential streams.
  - FIXED-PATTERN scatter (v2): the per-chunk one-hot of the previous version
    (DVE is_eq, 55us) is gone.  Per core, local destination rows are sorted
    by degree and dealt into 196 half-blocks of 64 rows; within half-block
    hb, chunk c holds TWO slots for row j (partitions j and j+64), so the
    scatter matrix of EVERY chunk is the same stacked identity PAT[p, j] =
    (p % 64 == j), loaded once as a [128, 64] bf16 input.  Row j's edges are
    dealt across (chunk, copy); capacity 2*R[hb] >= max deg in the block
    (degree sorting makes the max ~= the mean -> ~4% padding).
  - R[hb] is shared across cores (max) -> single SPMD program.  Chunk slot
    (ci, p) holds one edge; per-batch tensors are chunk-interleaved
    ([128, NB/ILV, d, ILV]) so DVE ops keep contiguous innermost APs
    (2x_1P mode) while PE operand slices keep an 8-byte stride.
    Per batch of NB chunks:
      * SWDGE DMA streams staged fp8-e4m3 x[col] rows, upconverting to
        bf16 in the DMA datapath (halves the dominant HBM stream)
      * msg = coef * x_col (bf16) -- the only per-edge DVE work
      * per chunk, one PE matmul accumulates into the owning pair's PSUM:
        psum[q*64 + j, 0:64] += sum_p PAT[p, j] * msg[p, :]
  - coef = sw * sigmoid(rep_row + rep_col) * ns_col is precomputed for ALL
    chunks in 4 bulk instructions at program start.  deg is pure layout
    metadata; the host supplies invdeg = 1/(deg + 1e-6) directly.
  - Per 128-row pair (two half-block runs share one [128, 64] PSUM tile):
    one ACT copy drains PSUM into a resident accumulator; every 14 pairs a
    grouped finalize does bulk agg*invdeg, sigmoid(rep), cat assembly, then
    per pair: PE transpose + one matmul with FIXED lhsT [W; W_self] giving
    the TRANSPOSED output (64-partition), ACT leaky-relu into a resident
    outT tile; one bulk DMA out at the end (host re-transposes).
"""
import sys

if "/opt/trn_rl_repo" not in sys.path:
    sys.path.insert(0, "/opt/trn_rl_repo")

import numpy as np

P = 128
D = 64
HALF = 64                              # rows per half-block / PAT width
QPP = P // HALF                        # half-blocks per 128-row output pair
N_NODES = 100000
N_CORES = 8
N_LOC = N_NODES // N_CORES             # 12500 destination rows per core
N_PAIR = (N_LOC + P - 1) // P          # 98 output pairs
N_HB = N_PAIR * QPP                    # 196 half-blocks (incl. virtual pad rows)
NB = 64                                # chunks per batch
ILV = 4                                # chunk interleave: PE operand stride 8B
GRP = 14                               # pairs per grouped finalize
# group boundaries: 14-pair groups, tail split finer to shorten the drain
# (all even: drains are batched two pairs per PSUM tile)
_BOUNDS = [0, 14, 28, 42, 56, 70, 84, 88, 92, 94, 96, 98]
GROUP_ENDS = {_BOUNDS[i + 1]: (_BOUNDS[i], _BOUNDS[i + 1] - _BOUNDS[i])
              for i in range(len(_BOUNDS) - 1)}
HB_PAD = N_HB                          # sentinel hb for tail-pad chunks (no MM)


def _layout(hcap, pad_chunks):
    """Chunk stream from per-half-block slot capacities (hcap[hb] is a
    multiple of P slots, shared across cores).  Within each 128-row pair the
    two runs (q0, q1) are INTERLEAVED chunk-by-chunk so consecutive matmuls
    target alternating PE column groups (tile_position 0 / 64) and can
    overlap in the array.  Trailing all-pad chunks emit no matmul."""
    chunk_meta = []        # per stream chunk: (hb, is_start, is_stop, pend)
    pos_of = {}            # (hb, chunk-in-run) -> stream index
    for pair in range(N_PAIR):
        h0, h1 = 2 * pair, 2 * pair + 1
        r0 = int(hcap[h0]) // P
        r1 = int(hcap[h1]) // P
        seq = [(h0, k, k == 0, k == r0 - 1) for k in range(r0)] + \
              [(h1, k, k == 0, k == r1 - 1) for k in range(r1)]
        for n, (hb, k, st, sp) in enumerate(seq):
            pos_of[(hb, k)] = len(chunk_meta)
            chunk_meta.append((hb, st, sp, n == len(seq) - 1))
    for _ in range(int(pad_chunks)):
        chunk_meta.append((HB_PAD, False, False, False))
    return pos_of, chunk_meta, len(chunk_meta)


def _build_program(hcap, pad_chunks):
    """Emit + compile the single-core SPMD program."""
    import concourse.bacc as bacc
    import concourse.mybir as mybir
    import concourse.tile as tile
    from concourse.masks import make_identity

    f32 = mybir.dt.float32
    bf16 = mybir.dt.bfloat16
    f8 = mybir.dt.float8e4

    _, chunk_meta, C = _layout(hcap, pad_chunks)

    nc = bacc.Bacc("TRN2", target_bir_lowering=False, debug=False)

    HEADB = 1          # leading batches staged bf16 (HWDGE, no Q7 wait)
    HEAD = 4 * NB      # chunks whose coef is computed in the prologue
    xg_d = nc.dram_tensor("xg", [P, C * D], f8, kind="ExternalInput")
    xgh_d = nc.dram_tensor("xg_head", [P, HEADB * NB * D], bf16,
                           kind="ExternalInput")
    # packed per-chunk metadata: [reprow, repc, sw, nsc] along dim 1
    meta_d = nc.dram_tensor("meta4", [P, 4 * C], bf16, kind="ExternalInput")
    invdeg_d = nc.dram_tensor("invdeg", [P, N_PAIR], bf16,
                              kind="ExternalInput")
    repsh_d = nc.dram_tensor("rep_sh", [P, N_PAIR], f32, kind="ExternalInput")
    xself_d = nc.dram_tensor("x_selfT", [P, N_PAIR * D], bf16,
                             kind="ExternalInput")
    pat_d = nc.dram_tensor("pat", [P, HALF], bf16, kind="ExternalInput")
    wcat_d = nc.dram_tensor("w_cat", [2 * D, D], bf16, kind="ExternalInput")
    out_d = nc.dram_tensor("out", [D, N_PAIR * P], bf16,
                           kind="ExternalOutput")

    AL = mybir.AluOpType
    ACT = mybir.ActivationFunctionType

    with tile.TileContext(nc) as tc:
        with (
            tc.tile_pool(name="meta", bufs=1) as meta,
            tc.tile_pool(name="gather", bufs=4) as gpool,
            tc.tile_pool(name="msgp", bufs=3) as mpool,
            tc.tile_pool(name="const", bufs=1) as cpool,
            tc.tile_pool(name="fin", bufs=16) as fpool,
            tc.tile_pool(name="psum", bufs=4, space="PSUM") as psum,
            tc.tile_pool(name="psumT", bufs=2, space="PSUM") as psumT,
        ):
            # meta HEAD lives in its own tile so the prologue coef pass only
            # depends on the small head DMA, not the 1.4MB tail DMA (tile-
            # granular dependency tracking)
            meta_h = meta.tile([P, 4, HEAD], bf16)
            meta_t = meta.tile([P, 4, C - HEAD], bf16)
            coefb = meta.tile([P, C], bf16)
            invdeg_s = meta.tile([P, N_PAIR], bf16)
            repsh_s = meta.tile([P, N_PAIR], f32)
            srep_all = meta.tile([P, N_PAIR], bf16)
            xselfb = meta.tile([P, N_PAIR, D], bf16)
            cat_all = meta.tile([P, N_PAIR, 2 * D], bf16)
            acc_all = meta.tile([P, N_PAIR, D], bf16)
            outsT = meta.tile([D, N_PAIR, P], bf16)
            wcat_s = cpool.tile([2 * D, D], bf16)
            ident = cpool.tile([P, P], bf16)
            pat_s = cpool.tile([P, HALF], bf16)
            # prepay the Q7 SWDGE ucode IRAM load (~6us) before batch 3's
            # cast-DMA needs it, overlapped with the prologue loads
            swdge_warm = cpool.tile([P, D], bf16)
            nc.gpsimd.dma_start(
                out=swdge_warm[:].rearrange("p d -> p d"),
                in_=xg_d[:, 0:D])
            # Prologue loads ride the SP (sync) HWDGE queue: the SP engine
            # is idle at startup while the scalar engine is busy with
            # activation-table loads, so DMA issue starts immediately.
            # DMA completion sems are per-queue COUNTERS: a consumer waits
            # for every DMA emitted on that queue so far.  So the small,
            # soon-needed inputs (pat, meta head) load first and the coef
            # head pass is emitted BEFORE the bulk tails are enqueued.
            # meta4 DRAM layout: [4*HEAD head block | 4*(C-HEAD) tail block],
            # both k-major and fully contiguous, so the head DMA is 128 big
            # descriptors instead of 512 strided ones
            nc.sync.dma_start(out=pat_s[:], in_=pat_d[:])
            nc.sync.dma_start(out=meta_h[:].rearrange("p k c -> p (k c)"),
                              in_=meta_d[:, :4 * HEAD])

            make_identity(nc, ident[:])

            # keep the PE clock gate (HAM) warm through the prologue
            warm_ps = psum.tile([P, 2, D], f32, tag="agg", name="warm_ps")
            for _ in range(40):
                nc.tensor.matmul(out=warm_ps[0:HALF, 0, :],
                                 lhsT=ident[:, 0:HALF],
                                 rhs=ident[:, 0:D],
                                 start=True, stop=True)

            # coef = sw * sigmoid(rep_row + rep_col) * ns_col. The head
            # slice is computed in the prologue; the tail pass is emitted
            # mid-loop (see below) so it does not block batches 0-2 in the
            # DVE instruction stream.
            def coef_pass(mt, lo, hi, off):
                nc.vector.tensor_tensor(out=coefb[:, off + lo:off + hi],
                                        in0=mt[:, 0, lo:hi],
                                        in1=mt[:, 1, lo:hi], op=AL.add)
                nc.scalar.activation(coefb[:, off + lo:off + hi],
                                     coefb[:, off + lo:off + hi],
                                     ACT.Sigmoid)
                nc.vector.tensor_tensor(out=coefb[:, off + lo:off + hi],
                                        in0=coefb[:, off + lo:off + hi],
                                        in1=mt[:, 2, lo:hi], op=AL.mult)
                nc.vector.tensor_tensor(out=coefb[:, off + lo:off + hi],
                                        in0=coefb[:, off + lo:off + hi],
                                        in1=mt[:, 3, lo:hi], op=AL.mult)

            coef_pass(meta_h, 0, HEAD, 0)

            # finalize-only inputs + bulk meta tail, AFTER the coef head
            # pass so its queue-counter wait does not cover them.  The xg
            # head batch (emitted below) precedes these on the sync queue.
            nc.sync.dma_start(out=invdeg_s[:], in_=invdeg_d[:])
            nc.sync.dma_start(out=repsh_s[:], in_=repsh_d[:])
            nc.sync.dma_start(out=wcat_s[:], in_=wcat_d[:])
            nc.scalar.dma_start(out=xselfb[:].rearrange("p b d -> p (b d)"),
                                in_=xself_d[:])
            nc.scalar.dma_start(out=meta_t[:].rearrange("p k c -> p (k c)"),
                                in_=meta_d[:, 4 * HEAD:])

            # the self-term half of cat does not depend on aggregation:
            # compute it once, DEPRIORITIZED and split into four pieces so
            # the scheduler slots them into DVE idle time instead of one
            # 3us blocker at the head of the DVE stream
            with tc.high_priority(offset=-2000000):
                nc.scalar.activation(srep_all[:], repsh_s[:], ACT.Sigmoid)
                SC = (N_PAIR + 3) // 4
                for s0 in range(0, N_PAIR, SC):
                    s1 = min(s0 + SC, N_PAIR)
                    nc.vector.tensor_tensor(
                        out=cat_all[:, s0:s1, D:2 * D],
                        in0=xselfb[:, s0:s1, :],
                        in1=srep_all[:, s0:s1]
                            .rearrange("p (b o) -> p b o", o=1)
                            .to_broadcast([P, s1 - s0, D]),
                        op=AL.mult)

            def finalize_group(lo, n, drain=False):
                # agg * invdeg on GPSIMD: the Pool engine is nearly idle and
                # this keeps the (drain-dependent) op out of the in-order
                # DVE FIFO where it would stall the msg stream
                nc.gpsimd.tensor_tensor(
                    out=cat_all[:, lo:lo + n, 0:D],
                    in0=acc_all[:, lo:lo + n, :],
                    in1=invdeg_s[:, lo:lo + n]
                        .rearrange("p (b o) -> p b o", o=1)
                        .to_broadcast([P, n, D]),
                    op=AL.mult)
                # software-pipelined across pairs: transpose k+1 is emitted
                # before matmul k so the PE never waits on the ACT copy;
                # leaky-relu drains two pairs per ACT op
                prev = None
                for k in range(n + 1):
                    if k < n:
                        pair = lo + k
                        ctp = psumT.tile([P, P], bf16, tag="ctp")
                        nc.tensor.transpose(out=ctp[:],
                                            in_=cat_all[:, pair, :],
                                            identity=ident[:])
                        catT = fpool.tile([P, P], bf16, tag="catT")
                        if drain:
                            # in the drain the DVE is idle while ACT
                            # serializes
                            nc.vector.tensor_copy(out=catT[:], in_=ctp[:])
                        else:
                            nc.scalar.copy(catT[:], ctp[:])
                    if prev is not None:
                        pc, pp = prev
                        if pp % 2 == 0:
                            ops2 = psumT.tile([HALF, 2, P], f32,
                                              tag="out_ps")
                        nc.tensor.matmul(out=ops2[:, pp % 2, :],
                                         lhsT=wcat_s[:],
                                         rhs=pc[:], start=True, stop=True)
                        if pp % 2 == 1:
                            nc.scalar.activation(
                                outsT[:, pp - 1:pp + 1, :]
                                    .rearrange("p b d -> p (b d)"),
                                ops2[:].rearrange("p b d -> p (b d)"),
                                ACT.Lrelu, alpha=0.01)
                    if k < n:
                        prev = (catT, pair)
                nc.scalar.dma_start(
                    out=out_d[:, lo * P:(lo + n) * P],
                    in_=outsT[:, lo:lo + n, :]
                        .rearrange("p b d -> p (b d)"))

            psum_cur = [None]
            pending = []   # finalize groups deferred to the next batch
            for bi, c0 in enumerate(range(0, C, NB)):
                xgb = gpool.tile([P, NB // ILV, D, ILV], bf16, tag="xg")
                if bi < HEADB:
                    # head batches: plain HWDGE load of bf16-staged data, so
                    # the stream starts before the Q7 SWDGE ucode is loaded
                    nc.sync.dma_start(
                        out=xgb[:].rearrange("p b d g -> p (b d g)"),
                        in_=xgh_d[:, c0 * D:(c0 + NB) * D])
                else:
                    # fp8 in HBM, upconverted to bf16 in the SWDGE datapath
                    nc.gpsimd.dma_start(
                        out=xgb[:].rearrange("p b d g -> p (b d g)"),
                        in_=xg_d[:, c0 * D:(c0 + NB) * D])

                msg = mpool.tile([P, NB // ILV, D, ILV], bf16, tag="msg")
                nc.vector.tensor_tensor(
                    out=msg[:], in0=xgb[:],
                    in1=coefb[:, c0:c0 + NB]
                        .rearrange("p (b o g) -> p b o g", o=1, g=ILV)
                        .to_broadcast([P, NB // ILV, D, ILV]),
                    op=AL.mult)

                if bi == 2:
                    # coef for batches 4+: deprioritized so the scheduler
                    # cannot place it ahead of the first batches' msg ops in
                    # the DVE stream (deps still force it before batch 4's
                    # msg multiply reads coefb)
                    with tc.high_priority(offset=-1000000):
                        coef_pass(meta_t, 0, C - HEAD, HEAD)

                # emit deferred finalize groups AFTER this batch's DVE prep:
                # their DVE/PE ops depend on earlier batches' matmuls, so
                # emitting them first would stall the DVE stream and starve
                # the PE of the next batch's msg
                for lo, n in pending:
                    finalize_group(lo, n)
                pending = []

                for i in range(NB):
                    hb, is_start, is_stop, is_pend = chunk_meta[c0 + i]
                    if hb == HB_PAD:
                        continue        # tail-pad chunk: no matmul
                    q = hb % QPP
                    pair = hb // QPP
                    # one PSUM tile holds TWO consecutive pairs so the
                    # drain copies move 128-wide rows (half the ACT copies)
                    if is_start and q == 0 and pair % 2 == 0:
                        psum_cur[0] = psum.tile([P, 2, D], f32, tag="agg",
                                                name="agg_ps")
                    ps = psum_cur[0]
                    nc.tensor.matmul(
                        out=ps[q * HALF:(q + 1) * HALF, pair % 2, :],
                        lhsT=pat_s[:],
                        rhs=msg[:, i // ILV, :, i % ILV],
                        start=is_start, stop=is_stop,
                        tile_position=(0, q * HALF))
                    if is_pend and pair % 2 == 1:
                        if c0 >= C - 2 * NB:
                            # drain region: ACT is the serializer, DVE idles
                            nc.vector.tensor_copy(
                                out=acc_all[:, pair - 1:pair + 1, :],
                                in_=ps[:])
                        else:
                            nc.scalar.copy(
                                acc_all[:, pair - 1:pair + 1, :], ps[:])
                        if pair + 1 in GROUP_ENDS:
                            # near the stream's end there is no later batch
                            # prep to protect; finalize eagerly to shorten
                            # the drain
                            if c0 >= C - 3 * NB:
                                finalize_group(*GROUP_ENDS[pair + 1],
                                               drain=True)
                            else:
                                pending.append(GROUP_ENDS[pair + 1])
            for lo, n in pending:
                finalize_group(lo, n, drain=True)

    nc.compile()
    return nc


def _preprocess(x, edge_index, sim_weight, rep, node_signal):
    """Host-side layout: degree-sort local rows into 64-row half-blocks,
    deal each row's edges across (chunk, copy) slots of the fixed stacked-
    identity pattern, stage per-slot arrays (including the x[col] rows) in
    stream order."""
    import ml_dtypes

    bf = ml_dtypes.bfloat16
    row = np.ascontiguousarray(edge_index[0]).astype(np.int64)
    col = np.ascontiguousarray(edge_index[1]).astype(np.int64)
    sw = np.ascontiguousarray(sim_weight).astype(np.float32)
    rep_f = np.ascontiguousarray(rep).astype(np.float32)
    ns_f = np.ascontiguousarray(node_signal).astype(np.float32)
    x_f = np.ascontiguousarray(x).astype(np.float32)
    E = row.shape[0]

    core = row // N_LOC
    lrow = row - core * N_LOC

    deg = np.zeros((N_CORES, N_LOC), dtype=np.int64)
    np.add.at(deg, (core, lrow), 1)

    # degree-sorted blocks of 64 rows: sorting clusters similar degrees so
    # each block's max degree ~= its mean -> minimal run padding.  Sorted
    # block b maps to physical half-block HBMAP[b], pairing block b with
    # block 195-b in one 128-row PSUM pair so heavy (early) and light
    # (late) runs alternate and pair completions spread evenly through the
    # chunk stream instead of clustering at its end.
    hbmap = np.empty(N_HB, dtype=np.int64)
    for b in range(N_HB):
        if b < N_PAIR:
            hbmap[b] = 2 * b                       # q0 of pair b
        else:
            hbmap[b] = 2 * (N_HB - 1 - b) + 1      # q1 of pair 195-b
    # rank k (in physical slot order): pair p, partition q*64+j  <-> sorted
    # block b with hbmap[b] = 2p+q, offset j
    inv_hbmap = np.argsort(hbmap)                  # physical hb -> sorted b
    rank = np.empty((N_CORES, N_LOC), dtype=np.int64)
    dmax_sorted = np.zeros((N_CORES, N_HB), dtype=np.int64)
    for c in range(N_CORES):
        order_r = np.argsort(-deg[c], kind="stable")
        srank = np.empty(N_LOC, dtype=np.int64)
        srank[order_r] = np.arange(N_LOC)          # sorted rank
        b = srank // HALF
        rank[c] = hbmap[b] * HALF + srank % HALF   # physical slot index
        ds = np.pad(deg[c][order_r], (0, N_HB * HALF - N_LOC))
        dmax_sorted[c] = ds.reshape(N_HB, HALF).max(axis=1)
    dmax = dmax_sorted[:, inv_hbmap]               # per physical hb

    # chunks per half-block run, shared across cores; capacity 2*R per row
    R = np.maximum(-(-dmax.max(axis=0) // 2), 1)
    hcap = R * P
    c_raw = int(R.sum())
    pad_chunks = (-c_raw) % NB          # all-pad tail chunks (no matmul)

    pos_of, _, C = _layout(hcap, pad_chunks)
    assert C % NB == 0
    Rmax = int(R.max())
    pos_arr = np.full((N_HB, Rmax), -1, dtype=np.int64)
    for (hb, k), s in pos_of.items():
        pos_arr[hb, k] = s
    total = C * P

    # intra-row edge index via stable sort on (core, local row)
    key = core * N_LOC + lrow
    order = np.argsort(key, kind="stable")
    gcounts = np.bincount(key, minlength=N_CORES * N_LOC)
    group_start = np.zeros(N_CORES * N_LOC + 1, dtype=np.int64)
    np.cumsum(gcounts, out=group_start[1:])
    i_intra = np.arange(E, dtype=np.int64) - group_start[key[order]]
    ko = key[order]
    core_o = ko // N_LOC
    rk = rank[core_o, ko % N_LOC]
    hb_o = rk // HALF
    j_o = rk % HALF
    cch = i_intra // 2                  # chunk within the run
    s_o = i_intra % 2                   # which stacked-identity copy
    gidx = (core_o * total + pos_arr[hb_o, cch] * P + j_o + HALF * s_o)

    tot = N_CORES * total
    sw_p = np.zeros(tot, dtype=np.float32)
    reprow_p = np.zeros(tot, dtype=np.float32)
    repc_p = np.zeros(tot, dtype=np.float32)
    nsc_p = np.zeros(tot, dtype=np.float32)
    sw_p[gidx] = sw[order]
    reprow_p[gidx] = rep_f[row[order]]
    repc_p[gidx] = rep_f[col[order]]
    nsc_p[gidx] = ns_f[col[order]]
    xg = np.zeros((tot, D), dtype=np.float32)
    xg[gidx] = x_f[col[order]]

    def per_core(a):
        return np.ascontiguousarray(
            a.reshape(N_CORES, C, P).transpose(0, 2, 1).astype(bf))

    sw_t = per_core(sw_p)
    reprow_t = per_core(reprow_p)
    repc_t = per_core(repc_p)
    nsc_t = per_core(nsc_p)

    # xg stream: per batch of NB chunks, [128, NB/ILV, D, ILV] interleaved so
    # the per-chunk PE operand stride is ILV elements
    NBG = NB // ILV
    xg16 = xg.astype(ml_dtypes.float8_e4m3).reshape(N_CORES, C, P, D)
    xgd = np.empty((N_CORES, P, C * D), dtype=ml_dtypes.float8_e4m3)
    for c0 in range(0, C, NB):
        blk = xg16[:, c0:c0 + NB].reshape(N_CORES, NBG, ILV, P, D)
        blk = blk.transpose(0, 3, 1, 4, 2)     # [8, 128, NBG, D, ILV]
        xgd[:, :, c0 * D:(c0 + NB) * D] = blk.reshape(N_CORES, P, NB * D)

    # finalize inputs, indexed by rank (slot_row): invdeg from layout counts
    inv_pad = np.zeros((N_CORES, N_PAIR * P), dtype=np.float32)
    rep_pad = np.zeros((N_CORES, N_PAIR * P), dtype=np.float32)
    xs_pad = np.zeros((N_CORES, N_PAIR * P, D), dtype=np.float32)
    for c in range(N_CORES):
        inv_pad[c, rank[c]] = 1.0 / (deg[c].astype(np.float32) + 1e-6)
        rep_pad[c, rank[c]] = rep_f[c * N_LOC:(c + 1) * N_LOC]
        xs_pad[c, rank[c]] = x_f[c * N_LOC:(c + 1) * N_LOC]
    invdeg_t = np.ascontiguousarray(
        inv_pad.reshape(N_CORES, N_PAIR, P).transpose(0, 2, 1).astype(bf))
    rep_sh = np.ascontiguousarray(
        rep_pad.reshape(N_CORES, N_PAIR, P).transpose(0, 2, 1))
    x_selfT = np.ascontiguousarray(
        xs_pad.reshape(N_CORES, N_PAIR, P, D).transpose(0, 2, 1, 3)
        .reshape(N_CORES, P, N_PAIR * D).astype(bf))

    pat = np.zeros((P, HALF), dtype=np.float32)
    pat[np.arange(P), np.arange(P) % HALF] = 1.0
    pat = np.ascontiguousarray(pat.astype(bf))

    xg_head = np.ascontiguousarray(xgd[:, :, :1 * NB * D].astype(bf))

    # [head block | tail block], each k-major contiguous (see device side)
    HEAD = 4 * NB
    stack = np.stack([reprow_t, repc_t, sw_t, nsc_t], axis=2)
    meta4 = np.ascontiguousarray(np.concatenate(
        [stack[:, :, :, :HEAD].reshape(N_CORES, P, 4 * HEAD),
         stack[:, :, :, HEAD:].reshape(N_CORES, P, 4 * (C - HEAD))],
        axis=2))

    return (hcap, pad_chunks, xgd, xg_head, meta4, invdeg_t, rep_sh,
            x_selfT, pat, rank)


_compiled = {}


def _get_program(hcap, pad_chunks):
    key = (tuple(hcap.tolist()), int(pad_chunks))
    if key not in _compiled:
        _compiled[key] = _build_program(hcap, pad_chunks)
    return _compiled[key]


def run(x, edge_index, sim_weight, rep, node_signal, W, W_self, trace=False):
    import ml_dtypes
    from concourse.bass_utils import run_bass_kernel_spmd

    (hcap, pad_chunks, xgd, xg_head, meta4, invdeg_t, rep_sh, x_selfT, pat,
     rank) = _preprocess(x, edge_index, sim_weight, rep, node_signal)
    w_cat = np.ascontiguousarray(
        np.concatenate([np.asarray(W, dtype=np.float32),
                        np.asarray(W_self, dtype=np.float32)],
                       axis=0).astype(ml_dtypes.bfloat16))
    nc = _get_program(hcap, pad_chunks)
    in_maps = []
    for c in range(N_CORES):
        in_maps.append({
            "xg": xgd[c],
            "xg_head": xg_head[c],
            "meta4": meta4[c],
            "invdeg": invdeg_t[c],
            "rep_sh": rep_sh[c],
            "x_selfT": x_selfT[c],
            "pat": pat,
            "w_cat": w_cat,
        })
    res = run_bass_kernel_spmd(nc, in_maps, core_ids=list(range(N_CORES)),
                               trace=trace)
    parts = []
    for c in range(N_CORES):
        o = res.results[c]["out"].astype(np.float32).reshape(D, N_PAIR, P)
        o = o.transpose(1, 2, 0).reshape(N_PAIR * P, D)
        parts.append(o[rank[c]])
    out = np.concatenate(parts, axis=0)
    return out, res


def kernel(x, edge_index, sim_weight, rep, node_signal, W, W_self):
    out, _ = run(x, edge_index, sim_weight, rep, node_signal, W, W_self)
    return out


# revision 37
# speedup vs baseline: 1.1344x; 1.1344x over previous
"""BehaviorAwareGCNLayer on 8 Trainium2 NeuronCores.

Math (reference):
    hx  = x @ W
    out[r] = (1/deg[r]) * sum_{e: row[e]=r} sim_w[e]*sigmoid(rep[row]+rep[col])*ns[col] * hx[col]
    out += sigmoid(rep) * (x @ W_self);  leaky_relu(out, 0.01)

Device strategy (destination sharding, no collectives):
  - By linearity, W is applied AFTER aggregation: agg[r] = sum coef_e * x[col_e],
    out[r] = (agg[r]/deg[r]) @ W + sigmoid(rep_r)*(x_r @ W_self).
  - Host does LAYOUT only (grouping/padding/fancy-index staging): it stages
    the per-edge x[col] rows and per-edge scalars (rep[row], rep[col], sw,
    ns[col]) into slot order so the device reads fully sequential streams.
  - FIXED-PATTERN scatter (v2): the per-chunk one-hot of the previous version
    (DVE is_eq, 55us) is gone.  Per core, local destination rows are sorted
    by degree and dealt into 196 half-blocks of 64 rows; within half-block
    hb, chunk c holds TWO slots for row j (partitions j and j+64), so the
    scatter matrix of EVERY chunk is the same stacked identity PAT[p, j] =
    (p % 64 == j), loaded once as a [128, 64] bf16 input.  Row j's edges are
    dealt across (chunk, copy); capacity 2*R[hb] >= max deg in the block
    (degree sorting makes the max ~= the mean -> ~4% padding).
  - R[hb] is shared across cores (max) -> single SPMD program.  Chunk slot
    (ci, p) holds one edge; per-batch tensors are chunk-interleaved
    ([128, NB/ILV, d, ILV]) so DVE ops keep contiguous innermost APs
    (2x_1P mode) while PE operand slices keep an 8-byte stride.
    Per batch of NB chunks:
      * SWDGE DMA streams staged fp8-e4m3 x[col] rows, upconverting to
        bf16 in the DMA datapath (halves the dominant HBM stream)
      * msg = coef * x_col (bf16) -- the only per-edge DVE work
      * per chunk, one PE matmul accumulates into the owning pair's PSUM:
        psum[q*64 + j, 0:64] += sum_p PAT[p, j] * msg[p, :]
  - coef = sw * sigmoid(rep_row + rep_col) * ns_col is precomputed for ALL
    chunks in 4 bulk instructions at program start.  deg is pure layout
    metadata; the host supplies invdeg = 1/(deg + 1e-6) directly.
  - Per 128-row pair (two half-block runs share one [128, 64] PSUM tile):
    one ACT copy drains PSUM into a resident accumulator; every 14 pairs a
    grouped finalize does bulk agg*invdeg, sigmoid(rep), cat assembly, then
    per pair: PE transpose + one matmul with FIXED lhsT [W; W_self] giving
    the TRANSPOSED output (64-partition), ACT leaky-relu into a resident
    outT tile; one bulk DMA out at the end (host re-transposes).
"""
import sys

if "/opt/trn_rl_repo" not in sys.path:
    sys.path.insert(0, "/opt/trn_rl_repo")

import numpy as np

P = 128
D = 64
HALF = 64                              # rows per half-block / PAT width
QPP = P // HALF                        # half-blocks per 128-row output pair
N_NODES = 100000
N_CORES = 8
N_LOC = N_NODES // N_CORES             # 12500 destination rows per core
N_PAIR = (N_LOC + P - 1) // P          # 98 output pairs
N_HB = N_PAIR * QPP                    # 196 half-blocks (incl. virtual pad rows)
NB = 64                                # chunks per batch
ILV = 4                                # chunk interleave: PE operand stride 8B
GRP = 14                               # pairs per grouped finalize
# group boundaries: 14-pair groups, tail split finer to shorten the drain
# (all even: drains are batched two pairs per PSUM tile)
_BOUNDS = [0, 14, 28, 42, 56, 70, 84, 88, 92, 94, 96, 98]
GROUP_ENDS = {_BOUNDS[i + 1]: (_BOUNDS[i], _BOUNDS[i + 1] - _BOUNDS[i])
              for i in range(len(_BOUNDS) - 1)}
HB_PAD = N_HB                          # sentinel hb for tail-pad chunks (no MM)


def _layout(hcap, pad_chunks):
    """Chunk stream from per-half-block slot capacities (hcap[hb] is a
    multiple of P slots, shared across cores).  Within each 128-row pair the
    two runs (q0, q1) are INTERLEAVED chunk-by-chunk so consecutive matmuls
    target alternating PE column groups (tile_position 0 / 64) and can
    overlap in the array.  Trailing all-pad chunks emit no matmul."""
    chunk_meta = []        # per stream chunk: (hb, is_start, is_stop, pend)
    pos_of = {}            # (hb, chunk-in-run) -> stream index
    for pair in range(N_PAIR):
        h0, h1 = 2 * pair, 2 * pair + 1
        r0 = int(hcap[h0]) // P
        r1 = int(hcap[h1]) // P
        seq = [(h0, k, k == 0, k == r0 - 1) for k in range(r0)] + \
              [(h1, k, k == 0, k == r1 - 1) for k in range(r1)]
        for n, (hb, k, st, sp) in enumerate(seq):
            pos_of[(hb, k)] = len(chunk_meta)
            chunk_meta.append((hb, st, sp, n == len(seq) - 1))
    for _ in range(int(pad_chunks)):
        chunk_meta.append((HB_PAD, False, False, False))
    return pos_of, chunk_meta, len(chunk_meta)


def _build_program(hcap, pad_chunks):
    """Emit + compile the single-core SPMD program."""
    import concourse.bacc as bacc
    import concourse.mybir as mybir
    import concourse.tile as tile
    from concourse.masks import make_identity

    f32 = mybir.dt.float32
    bf16 = mybir.dt.bfloat16
    f8 = mybir.dt.float8e4

    _, chunk_meta, C = _layout(hcap, pad_chunks)

    nc = bacc.Bacc("TRN2", target_bir_lowering=False, debug=False)

    HEADB = 1          # leading batches staged bf16 (HWDGE, no Q7 wait)
    HEAD = 4 * NB      # chunks whose coef is computed in the prologue
    xg_d = nc.dram_tensor("xg", [P, C * D], f8, kind="ExternalInput")
    xgh_d = nc.dram_tensor("xg_head", [P, HEADB * NB * D], bf16,
                           kind="ExternalInput")
    # packed per-chunk metadata: [reprow, repc, sw, nsc] along dim 1
    meta_d = nc.dram_tensor("meta4", [P, 4 * C], bf16, kind="ExternalInput")
    invdeg_d = nc.dram_tensor("invdeg", [P, N_PAIR], bf16,
                              kind="ExternalInput")
    repsh_d = nc.dram_tensor("rep_sh", [P, N_PAIR], f32, kind="ExternalInput")
    xself_d = nc.dram_tensor("x_selfT", [P, N_PAIR * D], bf16,
                             kind="ExternalInput")
    pat_d = nc.dram_tensor("pat", [P, HALF], bf16, kind="ExternalInput")
    wcat_d = nc.dram_tensor("w_cat", [2 * D, D], bf16, kind="ExternalInput")
    out_d = nc.dram_tensor("out", [D, N_PAIR * P], bf16,
                           kind="ExternalOutput")

    AL = mybir.AluOpType
    ACT = mybir.ActivationFunctionType

    with tile.TileContext(nc) as tc:
        with (
            tc.tile_pool(name="meta", bufs=1) as meta,
            tc.tile_pool(name="gather", bufs=4) as gpool,
            tc.tile_pool(name="msgp", bufs=3) as mpool,
            tc.tile_pool(name="const", bufs=1) as cpool,
            tc.tile_pool(name="fin", bufs=16) as fpool,
            tc.tile_pool(name="psum", bufs=4, space="PSUM") as psum,
            tc.tile_pool(name="psumT", bufs=2, space="PSUM") as psumT,
        ):
            # meta HEAD lives in its own tile so the prologue coef pass only
            # depends on the small head DMA, not the 1.4MB tail DMA (tile-
            # granular dependency tracking)
            meta_h = meta.tile([P, 4, HEAD], bf16)
            meta_t = meta.tile([P, 4, C - HEAD], bf16)
            coefb = meta.tile([P, C], bf16)
            invdeg_s = meta.tile([P, N_PAIR], bf16)
            repsh_s = meta.tile([P, N_PAIR], f32)
            srep_all = meta.tile([P, N_PAIR], bf16)
            xselfb = meta.tile([P, N_PAIR, D], bf16)
            cat_all = meta.tile([P, N_PAIR, 2 * D], bf16)
            acc_all = meta.tile([P, N_PAIR, D], bf16)
            outsT = meta.tile([D, N_PAIR, P], bf16)
            wcat_s = cpool.tile([2 * D, D], bf16)
            ident = cpool.tile([P, P], bf16)
            pat_s = cpool.tile([P, HALF], bf16)
            # prepay the Q7 SWDGE ucode IRAM load (~6us) before batch 3's
            # cast-DMA needs it, overlapped with the prologue loads
            swdge_warm = cpool.tile([P, D], bf16)
            nc.gpsimd.dma_start(
                out=swdge_warm[:].rearrange("p d -> p d"),
                in_=xg_d[:, 0:D])
            # Prologue loads ride the SP (sync) HWDGE queue: the SP engine
            # is idle at startup while the scalar engine is busy with
            # activation-table loads, so DMA issue starts immediately.
            # DMA completion sems are per-queue COUNTERS: a consumer waits
            # for every DMA emitted on that queue so far.  So the small,
            # soon-needed inputs (pat, meta head) load first and the coef
            # head pass is emitted BEFORE the bulk tails are enqueued.
            # meta4 DRAM layout: [4*HEAD head block | 4*(C-HEAD) tail block],
            # both k-major and fully contiguous, so the head DMA is 128 big
            # descriptors instead of 512 strided ones
            nc.sync.dma_start(out=pat_s[:], in_=pat_d[:])
            nc.sync.dma_start(out=meta_h[:].rearrange("p k c -> p (k c)"),
                              in_=meta_d[:, :4 * HEAD])

            make_identity(nc, ident[:])

            # keep the PE clock gate (HAM) warm through the prologue
            warm_ps = psum.tile([P, 2, 2, D], f32, tag="agg", name="warm_ps")
            for _ in range(40):
                nc.tensor.matmul(out=warm_ps[0:HALF, 0, 0, :],
                                 lhsT=ident[:, 0:HALF],
                                 rhs=ident[:, 0:D],
                                 start=True, stop=True)

            # coef = sw * sigmoid(rep_row + rep_col) * ns_col. The head
            # slice is computed in the prologue; the tail pass is emitted
            # mid-loop (see below) so it does not block batches 0-2 in the
            # DVE instruction stream.
            def coef_pass(mt, lo, hi, off):
                nc.vector.tensor_tensor(out=coefb[:, off + lo:off + hi],
                                        in0=mt[:, 0, lo:hi],
                                        in1=mt[:, 1, lo:hi], op=AL.add)
                nc.scalar.activation(coefb[:, off + lo:off + hi],
                                     coefb[:, off + lo:off + hi],
                                     ACT.Sigmoid)
                nc.vector.tensor_tensor(out=coefb[:, off + lo:off + hi],
                                        in0=coefb[:, off + lo:off + hi],
                                        in1=mt[:, 2, lo:hi], op=AL.mult)
                nc.vector.tensor_tensor(out=coefb[:, off + lo:off + hi],
                                        in0=coefb[:, off + lo:off + hi],
                                        in1=mt[:, 3, lo:hi], op=AL.mult)

            coef_pass(meta_h, 0, HEAD, 0)

            # finalize-only inputs + bulk meta tail, AFTER the coef head
            # pass so its queue-counter wait does not cover them.  The xg
            # head batch (emitted below) precedes these on the sync queue.
            nc.sync.dma_start(out=invdeg_s[:], in_=invdeg_d[:])
            nc.sync.dma_start(out=repsh_s[:], in_=repsh_d[:])
            nc.sync.dma_start(out=wcat_s[:], in_=wcat_d[:])
            nc.scalar.dma_start(out=xselfb[:].rearrange("p b d -> p (b d)"),
                                in_=xself_d[:])
            nc.scalar.dma_start(out=meta_t[:].rearrange("p k c -> p (k c)"),
                                in_=meta_d[:, 4 * HEAD:])

            # the self-term half of cat does not depend on aggregation:
            # compute it once, DEPRIORITIZED and split into four pieces so
            # the scheduler slots them into DVE idle time instead of one
            # 3us blocker at the head of the DVE stream
            with tc.high_priority(offset=-2000000):
                nc.scalar.activation(srep_all[:], repsh_s[:], ACT.Sigmoid)
                SC = (N_PAIR + 3) // 4
                for s0 in range(0, N_PAIR, SC):
                    s1 = min(s0 + SC, N_PAIR)
                    nc.vector.tensor_tensor(
                        out=cat_all[:, s0:s1, D:2 * D],
                        in0=xselfb[:, s0:s1, :],
                        in1=srep_all[:, s0:s1]
                            .rearrange("p (b o) -> p b o", o=1)
                            .to_broadcast([P, s1 - s0, D]),
                        op=AL.mult)

            def finalize_group(lo, n, drain=False):
                # agg * invdeg on GPSIMD: the Pool engine is nearly idle and
                # this keeps the (drain-dependent) op out of the in-order
                # DVE FIFO where it would stall the msg stream
                nc.gpsimd.tensor_tensor(
                    out=cat_all[:, lo:lo + n, 0:D],
                    in0=acc_all[:, lo:lo + n, :],
                    in1=invdeg_s[:, lo:lo + n]
                        .rearrange("p (b o) -> p b o", o=1)
                        .to_broadcast([P, n, D]),
                    op=AL.mult)
                # software-pipelined across pairs: transpose k+1 is emitted
                # before matmul k so the PE never waits on the ACT copy;
                # leaky-relu drains two pairs per ACT op
                prev = None
                for k in range(n + 1):
                    if k < n:
                        pair = lo + k
                        ctp = psumT.tile([P, P], bf16, tag="ctp")
                        nc.tensor.transpose(out=ctp[:],
                                            in_=cat_all[:, pair, :],
                                            identity=ident[:])
                        catT = fpool.tile([P, P], bf16, tag="catT")
                        if drain:
                            # in the drain the DVE is idle while ACT
                            # serializes
                            nc.vector.tensor_copy(out=catT[:], in_=ctp[:])
                        else:
                            nc.scalar.copy(catT[:], ctp[:])
                    if prev is not None:
                        pc, pp = prev
                        if pp % 2 == 0:
                            ops2 = psumT.tile([HALF, 2, P], f32,
                                              tag="out_ps")
                        nc.tensor.matmul(out=ops2[:, pp % 2, :],
                                         lhsT=wcat_s[:],
                                         rhs=pc[:], start=True, stop=True)
                        if pp % 2 == 1:
                            nc.scalar.activation(
                                outsT[:, pp - 1:pp + 1, :]
                                    .rearrange("p b d -> p (b d)"),
                                ops2[:].rearrange("p b d -> p (b d)"),
                                ACT.Lrelu, alpha=0.01)
                    if k < n:
                        prev = (catT, pair)
                nc.scalar.dma_start(
                    out=out_d[:, lo * P:(lo + n) * P],
                    in_=outsT[:, lo:lo + n, :]
                        .rearrange("p b d -> p (b d)"))

            # Pair consecutive chunks of the same run into one N=128 matmul
            # (k column = stream parity, so even-indexed pairs never cross
            # an ILV group or batch boundary).  Edge chunks at run
            # boundaries stay single; per-element has_written makes the
            # region-cleared but unwritten k column overwrite-on-first-
            # write, and every run has >= 2 chunks so both k columns get
            # written.  The drain then adds the two k columns on the DVE.
            mms = {}       # stream index -> (nk, hb, start, stop, pend)
            ci = 0
            NCM = len(chunk_meta)
            while ci < NCM:
                hb, st, sp, pe_ = chunk_meta[ci]
                if hb == HB_PAD:
                    ci += 1
                    continue
                if ci % 2 == 0 and ci + 1 < NCM and chunk_meta[ci + 1][0] == hb:
                    _, _, sp2, pe2 = chunk_meta[ci + 1]
                    mms[ci] = (2, hb, st, sp2, pe2)
                    ci += 2
                else:
                    mms[ci] = (1, hb, st, sp, pe_)
                    ci += 1

            psum_cur = [None]
            pending = []   # finalize groups deferred to the next batch
            for bi, c0 in enumerate(range(0, C, NB)):
                xgb = gpool.tile([P, NB // ILV, D, ILV], bf16, tag="xg")
                if bi < HEADB:
                    # head batches: plain HWDGE load of bf16-staged data, so
                    # the stream starts before the Q7 SWDGE ucode is loaded
                    nc.sync.dma_start(
                        out=xgb[:].rearrange("p b d g -> p (b d g)"),
                        in_=xgh_d[:, c0 * D:(c0 + NB) * D])
                else:
                    # fp8 in HBM, upconverted to bf16 in the SWDGE datapath
                    nc.gpsimd.dma_start(
                        out=xgb[:].rearrange("p b d g -> p (b d g)"),
                        in_=xg_d[:, c0 * D:(c0 + NB) * D])

                msg = mpool.tile([P, NB // ILV, D, ILV], bf16, tag="msg")
                nc.vector.tensor_tensor(
                    out=msg[:], in0=xgb[:],
                    in1=coefb[:, c0:c0 + NB]
                        .rearrange("p (b o g) -> p b o g", o=1, g=ILV)
                        .to_broadcast([P, NB // ILV, D, ILV]),
                    op=AL.mult)

                if bi == 2:
                    # coef for batches 4+: deprioritized so the scheduler
                    # cannot place it ahead of the first batches' msg ops in
                    # the DVE stream (deps still force it before batch 4's
                    # msg multiply reads coefb)
                    with tc.high_priority(offset=-1000000):
                        coef_pass(meta_t, 0, C - HEAD, HEAD)

                # emit deferred finalize groups AFTER this batch's DVE prep:
                # their DVE/PE ops depend on earlier batches' matmuls, so
                # emitting them first would stall the DVE stream and starve
                # the PE of the next batch's msg
                for lo, n in pending:
                    finalize_group(lo, n)
                pending = []

                for i in range(NB):
                    gi = c0 + i
                    mm = mms.get(gi)
                    if mm is None:
                        continue        # pad chunk or covered by a pair
                    nk, hb, is_start, is_stop, is_pend = mm
                    q = hb % QPP
                    pair = hb // QPP
                    # one PSUM tile holds TWO consecutive pairs x two k
                    # columns; the drain adds the k columns on the DVE
                    if is_start and q == 0 and pair % 2 == 0:
                        psum_cur[0] = psum.tile([P, 2, 2, D], f32,
                                                tag="agg", name="agg_ps")
                    ps = psum_cur[0]
                    if nk == 2:
                        rhs = msg[:, i // ILV, :, i % ILV:i % ILV + 2]
                        outap = (ps[q * HALF:(q + 1) * HALF, pair % 2, :, :]
                                 .rearrange("p k d -> p d k"))
                    else:
                        rhs = msg[:, i // ILV, :, i % ILV]
                        outap = ps[q * HALF:(q + 1) * HALF, pair % 2,
                                   gi % 2, :]
                    nc.tensor.matmul(
                        out=outap, lhsT=pat_s[:], rhs=rhs,
                        start=is_start, stop=is_stop,
                        tile_position=(0, q * HALF))
                    if is_pend and pair % 2 == 1:
                        nc.vector.tensor_tensor(
                            out=acc_all[:, pair - 1:pair + 1, :],
                            in0=ps[:, :, 0, :], in1=ps[:, :, 1, :],
                            op=AL.add)
                        if pair + 1 in GROUP_ENDS:
                            # near the stream's end there is no later batch
                            # prep to protect; finalize eagerly to shorten
                            # the drain
                            if c0 >= C - 3 * NB:
                                finalize_group(*GROUP_ENDS[pair + 1],
                                               drain=True)
                            else:
                                pending.append(GROUP_ENDS[pair + 1])
            for lo, n in pending:
                finalize_group(lo, n, drain=True)

    nc.compile()
    return nc


def _preprocess(x, edge_index, sim_weight, rep, node_signal):
    """Host-side layout: degree-sort local rows into 64-row half-blocks,
    deal each row's edges across (chunk, copy) slots of the fixed stacked-
    identity pattern, stage per-slot arrays (including the x[col] rows) in
    stream order."""
    import ml_dtypes

    bf = ml_dtypes.bfloat16
    row = np.ascontiguousarray(edge_index[0]).astype(np.int64)
    col = np.ascontiguousarray(edge_index[1]).astype(np.int64)
    sw = np.ascontiguousarray(sim_weight).astype(np.float32)
    rep_f = np.ascontiguousarray(rep).astype(np.float32)
    ns_f = np.ascontiguousarray(node_signal).astype(np.float32)
    x_f = np.ascontiguousarray(x).astype(np.float32)
    E = row.shape[0]

    core = row // N_LOC
    lrow = row - core * N_LOC

    deg = np.zeros((N_CORES, N_LOC), dtype=np.int64)
    np.add.at(deg, (core, lrow), 1)

    # degree-sorted blocks of 64 rows: sorting clusters similar degrees so
    # each block's max degree ~= its mean -> minimal run padding.  Sorted
    # block b maps to physical half-block HBMAP[b], pairing block b with
    # block 195-b in one 128-row PSUM pair so heavy (early) and light
    # (late) runs alternate and pair completions spread evenly through the
    # chunk stream instead of clustering at its end.
    hbmap = np.empty(N_HB, dtype=np.int64)
    for b in range(N_HB):
        if b < N_PAIR:
            hbmap[b] = 2 * b                       # q0 of pair b
        else:
            hbmap[b] = 2 * (N_HB - 1 - b) + 1      # q1 of pair 195-b
    # rank k (in physical slot order): pair p, partition q*64+j  <-> sorted
    # block b with hbmap[b] = 2p+q, offset j
    inv_hbmap = np.argsort(hbmap)                  # physical hb -> sorted b
    rank = np.empty((N_CORES, N_LOC), dtype=np.int64)
    dmax_sorted = np.zeros((N_CORES, N_HB), dtype=np.int64)
    for c in range(N_CORES):
        order_r = np.argsort(-deg[c], kind="stable")
        srank = np.empty(N_LOC, dtype=np.int64)
        srank[order_r] = np.arange(N_LOC)          # sorted rank
        b = srank // HALF
        rank[c] = hbmap[b] * HALF + srank % HALF   # physical slot index
        ds = np.pad(deg[c][order_r], (0, N_HB * HALF - N_LOC))
        dmax_sorted[c] = ds.reshape(N_HB, HALF).max(axis=1)
    dmax = dmax_sorted[:, inv_hbmap]               # per physical hb

    # chunks per half-block run, shared across cores; capacity 2*R per row
    R = np.maximum(-(-dmax.max(axis=0) // 2), 1)
    hcap = R * P
    c_raw = int(R.sum())
    pad_chunks = (-c_raw) % NB          # all-pad tail chunks (no matmul)

    pos_of, _, C = _layout(hcap, pad_chunks)
    assert C % NB == 0
    Rmax = int(R.max())
    pos_arr = np.full((N_HB, Rmax), -1, dtype=np.int64)
    for (hb, k), s in pos_of.items():
        pos_arr[hb, k] = s
    total = C * P

    # intra-row edge index via stable sort on (core, local row)
    key = core * N_LOC + lrow
    order = np.argsort(key, kind="stable")
    gcounts = np.bincount(key, minlength=N_CORES * N_LOC)
    group_start = np.zeros(N_CORES * N_LOC + 1, dtype=np.int64)
    np.cumsum(gcounts, out=group_start[1:])
    i_intra = np.arange(E, dtype=np.int64) - group_start[key[order]]
    ko = key[order]
    core_o = ko // N_LOC
    rk = rank[core_o, ko % N_LOC]
    hb_o = rk // HALF
    j_o = rk % HALF
    cch = i_intra // 2                  # chunk within the run
    s_o = i_intra % 2                   # which stacked-identity copy
    gidx = (core_o * total + pos_arr[hb_o, cch] * P + j_o + HALF * s_o)

    tot = N_CORES * total
    sw_p = np.zeros(tot, dtype=np.float32)
    reprow_p = np.zeros(tot, dtype=np.float32)
    repc_p = np.zeros(tot, dtype=np.float32)
    nsc_p = np.zeros(tot, dtype=np.float32)
    sw_p[gidx] = sw[order]
    reprow_p[gidx] = rep_f[row[order]]
    repc_p[gidx] = rep_f[col[order]]
    nsc_p[gidx] = ns_f[col[order]]
    xg = np.zeros((tot, D), dtype=np.float32)
    xg[gidx] = x_f[col[order]]

    def per_core(a):
        return np.ascontiguousarray(
            a.reshape(N_CORES, C, P).transpose(0, 2, 1).astype(bf))

    sw_t = per_core(sw_p)
    reprow_t = per_core(reprow_p)
    repc_t = per_core(repc_p)
    nsc_t = per_core(nsc_p)

    # xg stream: per batch of NB chunks, [128, NB/ILV, D, ILV] interleaved so
    # the per-chunk PE operand stride is ILV elements
    NBG = NB // ILV
    xg16 = xg.astype(ml_dtypes.float8_e4m3).reshape(N_CORES, C, P, D)
    xgd = np.empty((N_CORES, P, C * D), dtype=ml_dtypes.float8_e4m3)
    for c0 in range(0, C, NB):
        blk = xg16[:, c0:c0 + NB].reshape(N_CORES, NBG, ILV, P, D)
        blk = blk.transpose(0, 3, 1, 4, 2)     # [8, 128, NBG, D, ILV]
        xgd[:, :, c0 * D:(c0 + NB) * D] = blk.reshape(N_CORES, P, NB * D)

    # finalize inputs, indexed by rank (slot_row): invdeg from layout counts
    inv_pad = np.zeros((N_CORES, N_PAIR * P), dtype=np.float32)
    rep_pad = np.zeros((N_CORES, N_PAIR * P), dtype=np.float32)
    xs_pad = np.zeros((N_CORES, N_PAIR * P, D), dtype=np.float32)
    for c in range(N_CORES):
        inv_pad[c, rank[c]] = 1.0 / (deg[c].astype(np.float32) + 1e-6)
        rep_pad[c, rank[c]] = rep_f[c * N_LOC:(c + 1) * N_LOC]
        xs_pad[c, rank[c]] = x_f[c * N_LOC:(c + 1) * N_LOC]
    invdeg_t = np.ascontiguousarray(
        inv_pad.reshape(N_CORES, N_PAIR, P).transpose(0, 2, 1).astype(bf))
    rep_sh = np.ascontiguousarray(
        rep_pad.reshape(N_CORES, N_PAIR, P).transpose(0, 2, 1))
    x_selfT = np.ascontiguousarray(
        xs_pad.reshape(N_CORES, N_PAIR, P, D).transpose(0, 2, 1, 3)
        .reshape(N_CORES, P, N_PAIR * D).astype(bf))

    pat = np.zeros((P, HALF), dtype=np.float32)
    pat[np.arange(P), np.arange(P) % HALF] = 1.0
    pat = np.ascontiguousarray(pat.astype(bf))

    xg_head = np.ascontiguousarray(xgd[:, :, :1 * NB * D].astype(bf))

    # [head block | tail block], each k-major contiguous (see device side)
    HEAD = 4 * NB
    stack = np.stack([reprow_t, repc_t, sw_t, nsc_t], axis=2)
    meta4 = np.ascontiguousarray(np.concatenate(
        [stack[:, :, :, :HEAD].reshape(N_CORES, P, 4 * HEAD),
         stack[:, :, :, HEAD:].reshape(N_CORES, P, 4 * (C - HEAD))],
        axis=2))

    return (hcap, pad_chunks, xgd, xg_head, meta4, invdeg_t, rep_sh,
            x_selfT, pat, rank)


_compiled = {}


def _get_program(hcap, pad_chunks):
    key = (tuple(hcap.tolist()), int(pad_chunks))
    if key not in _compiled:
        _compiled[key] = _build_program(hcap, pad_chunks)
    return _compiled[key]


def run(x, edge_index, sim_weight, rep, node_signal, W, W_self, trace=False):
    import ml_dtypes
    from concourse.bass_utils import run_bass_kernel_spmd

    (hcap, pad_chunks, xgd, xg_head, meta4, invdeg_t, rep_sh, x_selfT, pat,
     rank) = _preprocess(x, edge_index, sim_weight, rep, node_signal)
    w_cat = np.ascontiguousarray(
        np.concatenate([np.asarray(W, dtype=np.float32),
                        np.asarray(W_self, dtype=np.float32)],
                       axis=0).astype(ml_dtypes.bfloat16))
    nc = _get_program(hcap, pad_chunks)
    in_maps = []
    for c in range(N_CORES):
        in_maps.append({
            "xg": xgd[c],
            "xg_head": xg_head[c],
            "meta4": meta4[c],
            "invdeg": invdeg_t[c],
            "rep_sh": rep_sh[c],
            "x_selfT": x_selfT[c],
            "pat": pat,
            "w_cat": w_cat,
        })
    res = run_bass_kernel_spmd(nc, in_maps, core_ids=list(range(N_CORES)),
                               trace=trace)
    parts = []
    for c in range(N_CORES):
        o = res.results[c]["out"].astype(np.float32).reshape(D, N_PAIR, P)
        o = o.transpose(1, 2, 0).reshape(N_PAIR * P, D)
        parts.append(o[rank[c]])
    out = np.concatenate(parts, axis=0)
    return out, res


def kernel(x, edge_index, sim_weight, rep, node_signal, W, W_self):
    out, _ = run(x, edge_index, sim_weight, rep, node_signal, W, W_self)
    return out
